# revision 1
# baseline (speedup 1.0000x reference)
"""Trainium2 Bass kernel for all-pairs log-polar repulsion (gnn_message_passing).

Math: the reference's log-space distance chain collapses in linear space:
  exp(-ld) = 1/sqrt(dx^2+dy^2)  with x = r*(cos t + EPS*sign(cos t)), etc.
Row-sharded over 8 cores (512 query rows each). Each core streams 32 j-chunks
of 128 nodes; per chunk computes a [128j x 512i] force tile and reduces over j
with PE matmuls into PSUM:
  out0 = sum_j s_j*g_ij, out1 = sum_j s_j*g_ij*ell_j, out2 = sum_j s_j*g_ij*th_j,
  outq = sum_j s_j*g_ij*([tmp>=tau] - [tmp<0])   (exact jnp.mod wrap indicators)
Host assembles F_ell = s_i*(out1 - ell_i*out0), F_th = s_i*(out2 - th_i*out0 - tau*outq).
j-chunks are permuted per core so the 4 diagonal blocks are always local chunks
0..3 (processed last); their self-pairs are zeroed with a shifted-window mask.
"""

import sys

sys.path.insert(0, "/opt/trn_rl_repo")

from contextlib import ExitStack

import numpy as np

import concourse.bass as bass
import concourse.mybir as mybir
import concourse.tile as tile

N = 4096
NCORES = 8
IPC = N // NCORES  # 512 rows per core
NJC = N // 128  # 32 j-chunks of 128
EPS = np.float32(1e-10)
PHI = (1.0 + np.sqrt(5.0)) / 2.0
TAU32 = float(np.float32(2.0 * np.pi))
PI32 = float(np.float32(np.pi))
CUT2 = float(np.float32(PHI**4))  # dist^2 cutoff = phi^4
D2MIN = 1e-20

# "dsqrt": force = 2*Dsqrt(d2) on ACT (1 op). "recip": rsqrt via DVE
# reciprocal_approx_fast + ACT Sqrt (2 ops). Host FACT undoes the 1/2.
VARIANT = "recip"

_cache = {}


def _build(variant=VARIANT):
    f32 = mybir.dt.float32
    AF = mybir.ActivationFunctionType
    OP = mybir.AluOpType
    nc = bass.Bass()

    # every per-core input packed in ONE tensor -> one DMA, one semaphore
    NALL = 8 * NJC + 896 + 3 * IPC
    d_all = nc.declare_dram_parameter("allin", [128, NALL], f32, isOutput=False)
    d_out = nc.declare_dram_parameter("out", [4, IPC], f32, isOutput=True)

    with tile.TileContext(nc) as tc, ExitStack() as ctx:
        const = ctx.enter_context(tc.tile_pool(name="const", bufs=1))
        work = ctx.enter_context(tc.tile_pool(name="work", bufs=3))
        psum = ctx.enter_context(tc.tile_pool(name="psum", bufs=1, space="PSUM"))

        t_all = const.tile([128, NALL], f32)
        nc.gpsimd.dma_start(t_all[:], d_all[:])
        t_negx = t_all[:, 0:NJC]
        t_negy = t_all[:, NJC : 2 * NJC]
        t_thj = t_all[:, 2 * NJC : 3 * NJC]
        t_sp = t_all[:, 3 * NJC : 4 * NJC]
        t_sm = t_all[:, 4 * NJC : 5 * NJC]
        t_w3 = t_all[:, 5 * NJC : 8 * NJC]
        o = 8 * NJC
        t_dmask = t_all[:, o : o + 896]
        xrow = t_all[:, o + 896 : o + 896 + IPC]
        yrow = t_all[:, o + 896 + IPC : o + 896 + 2 * IPC]
        thrm = t_all[:, o + 896 + 2 * IPC : o + 896 + 3 * IPC]

        psum3 = psum.tile([3, IPC], f32)
        psumq = psum.tile([1, IPC], f32)

        # warmups: absorb the input-DMA wait on PE/GPS before the hot loop so
        # steady-state instructions carry at most one sync wait each.
        wps = psum.tile([1, 4], f32)
        nc.tensor.matmul(wps[:], t_all[:, 0:1], t_all[:, 0:4], start=True, stop=True)
        wgs = work.tile([128, 1], f32)
        nc.gpsimd.tensor_scalar(wgs[:], t_all[:, 0:1], 0.0, None, op0=OP.add)

        # diagonal chunks (local 0..3) last so the dmask DMA has time to land
        order = list(range(4, NJC)) + [0, 1, 2, 3]
        for idx, c in enumerate(order):
            first, last = idx == 0, idx == NJC - 1
            sqx = work.tile([128, IPC], f32)
            nc.scalar.activation(sqx[:], xrow[:], AF.Square, bias=t_negx[:, c : c + 1])
            sqy = work.tile([128, IPC], f32)
            nc.scalar.activation(sqy[:], yrow[:], AF.Square, bias=t_negy[:, c : c + 1])
            d2 = work.tile([128, IPC], f32)
            nc.vector.scalar_tensor_tensor(
                d2[:], sqx[:], D2MIN, sqy[:], op0=OP.max, op1=OP.add
            )
            f = work.tile([128, IPC], f32)
            if variant == "dsqrt":
                nc.scalar.activation(f[:], d2[:], AF.Dsqrt)
            else:
                # rsqrt(d2) = exp(-0.5*ln(d2)) with standard ACT funcs
                ln = work.tile([128, IPC], f32)
                nc.scalar.activation(ln[:], d2[:], AF.Ln)
                nc.scalar.activation(f[:], ln[:], AF.Exp, scale=-0.5)
            g = work.tile([128, IPC], f32)
            nc.vector.scalar_tensor_tensor(
                g[:], d2[:], CUT2, f[:], op0=OP.is_le, op1=OP.mult
            )
            if c < 4:  # zero the self-pair diagonal of this block
                g2 = work.tile([128, IPC], f32)
                nc.gpsimd.tensor_tensor(
                    g2[:], g[:], t_dmask[:, 384 - 128 * c : 896 - 128 * c], op=OP.mult
                )
                g = g2
            tmp = work.tile([128, IPC], f32)
            nc.gpsimd.tensor_scalar(
                tmp[:], thrm[:], t_thj[:, c : c + 1], PI32, op0=OP.add, op1=OP.add
            )
            P = work.tile([128, IPC], f32)
            nc.vector.scalar_tensor_tensor(
                P[:], tmp[:], TAU32, g[:], op0=OP.is_ge, op1=OP.mult
            )
            M = work.tile([128, IPC], f32)
            nc.vector.scalar_tensor_tensor(
                M[:], tmp[:], 0.0, g[:], op0=OP.is_lt, op1=OP.mult
            )
            nc.tensor.matmul(
                psum3[:], t_w3[:, 3 * c : 3 * c + 3], g[:], start=first, stop=last
            )
            nc.tensor.matmul(
                psumq[:], t_sp[:, c : c + 1], P[:], start=first, stop=False
            )
            nc.tensor.matmul(
                psumq[:], t_sm[:, c : c + 1], M[:], start=False, stop=last
            )

        outt3 = work.tile([3, IPC], f32)
        nc.vector.tensor_copy(outt3[:], psum3[:])
        outtq = work.tile([1, IPC], f32)
        nc.vector.tensor_copy(outtq[:], psumq[:])
        nc.gpsimd.dma_start(d_out[0:3, :], outt3[:])
        nc.gpsimd.dma_start(d_out[3:4, :], outtq[:])
    return nc


def _host_prep(ell, theta, s, frozen):
    f32 = np.float32
    ell = np.asarray(ell, f32)
    theta = np.asarray(theta, f32)
    s = np.asarray(s, f32)
    c = np.cos(theta).astype(f32)
    sn = np.sin(theta).astype(f32)
    r = np.exp(ell).astype(f32)
    x = (r * (c + EPS * np.sign(c))).astype(f32)
    y = (r * (sn + EPS * np.sign(sn))).astype(f32)

    def cols(a):  # [N] -> [128, NJC], chunk c in column c
        return np.ascontiguousarray(a.reshape(NJC, 128).T)

    xc, yc, thc = cols(x), cols(y), cols(theta)
    sc, ec = cols(s), cols(ell)
    w3 = np.stack([sc, sc * ec, sc * thc], axis=2)  # [128, NJC, 3]
    dmask = np.ones((128, 896), f32)
    dmask[np.arange(128), 384 + np.arange(128)] = 0.0

    in_maps = []
    for k in range(NCORES):
        perm = [(cc + 4 * k) % NJC for cc in range(NJC)]
        sl = slice(k * IPC, (k + 1) * IPC)
        in_maps.append(
            {
                "allin": np.ascontiguousarray(
                    np.concatenate(
                        [
                            -xc[:, perm],
                            -yc[:, perm],
                            thc[:, perm],
                            sc[:, perm],
                            -sc[:, perm],
                            w3[:, perm, :].reshape(128, 3 * NJC),
                            dmask,
                            np.broadcast_to(x[sl], (128, IPC)),
                            np.broadcast_to(y[sl], (128, IPC)),
                            np.broadcast_to(-theta[sl], (128, IPC)),
                        ],
                        axis=1,
                    )
                ),
            }
        )
    return in_maps


def _assemble(ell, theta, s, frozen, outs, variant=VARIANT):
    fact = 2.0 if variant == "dsqrt" else 1.0
    ell64 = np.asarray(ell, np.float64)
    th64 = np.asarray(theta, np.float64)
    s64 = np.asarray(s, np.float64)
    nf = 1.0 - np.asarray(frozen, np.float64)
    Fe = np.empty(N)
    Ft = np.empty(N)
    for k in range(NCORES):
        sl = slice(k * IPC, (k + 1) * IPC)
        o = np.asarray(outs[k], np.float64) * fact
        Fe[sl] = o[1] - ell64[sl] * o[0]
        Ft[sl] = o[2] - th64[sl] * o[0] - 2.0 * np.pi * o[3]
    Fe *= s64 * nf
    Ft *= s64 * nf
    return np.stack([Fe, Ft]).astype(np.float32)


def run_device(ell, theta, s, frozen, trace=False, variant=VARIANT):
    from concourse.bass_utils import run_bass_kernel_spmd

    key = ("nc", variant)
    if key not in _cache:
        _cache[key] = _build(variant)
    nc = _cache[key]
    in_maps = _host_prep(ell, theta, s, frozen)
    res = run_bass_kernel_spmd(
        nc, in_maps, list(range(NCORES)), trace=trace, trace_cores=[0]
    )
    outs = [res.results[k]["out"] for k in range(NCORES)]
    return _assemble(ell, theta, s, frozen, outs, variant), res


_jax_cache = {}


def _jax_kernel():
    if "fn" in _jax_cache:
        return _jax_cache["fn"]
    import jax
    import jax.numpy as jnp

    f32 = jnp.float32
    CUT2j = f32(np.float32(PHI**4))
    TAUj = f32(np.float32(2.0 * np.pi))
    PIj = f32(np.float32(np.pi))

    def per_core(i0, x, y, th, ell, sj):
        # i0: scalar row offset; computes F for rows [i0, i0+IPC)
        idx = i0 + jnp.arange(IPC)
        xi = x[idx]
        yi = y[idx]
        ti = th[idx]
        ei = ell[idx]
        dx = xi[:, None] - x[None, :]
        dy = yi[:, None] - y[None, :]
        d2 = dx * dx + dy * dy
        notdiag = (idx[:, None] != jnp.arange(N)[None, :]).astype(f32)
        g = (d2 <= CUT2j).astype(f32) * notdiag * sj[None, :]
        g = g / jnp.sqrt(jnp.maximum(d2, f32(1e-20)))
        tmp = (th[None, :] - ti[:, None]) + PIj
        dth = (th[None, :] - ti[:, None]) - TAUj * (tmp >= TAUj).astype(
            f32
        ) + TAUj * (tmp < 0).astype(f32)
        de = ell[None, :] - ei[:, None]
        return jnp.stack([(g * de).sum(1), (g * dth).sum(1)])

    pm = jax.pmap(per_core, in_axes=(0, None, None, None, None, None))
    _jax_cache["fn"] = pm
    return pm


def kernel(ell, theta, s, frozen):
    f32 = np.float32
    ell32 = np.asarray(ell, f32)
    theta32 = np.asarray(theta, f32)
    s32 = np.asarray(s, f32)
    c = np.cos(theta32).astype(f32)
    sn = np.sin(theta32).astype(f32)
    r = np.exp(ell32).astype(f32)
    x = (r * (c + EPS * np.sign(c))).astype(f32)
    y = (r * (sn + EPS * np.sign(sn))).astype(f32)
    pm = _jax_kernel()
    i0s = np.arange(NCORES, dtype=np.int32) * IPC
    out = np.asarray(pm(i0s, x, y, theta32, ell32, s32))  # [8, 2, 512]
    F = np.concatenate([out[k] for k in range(NCORES)], axis=1)
    F = F * (s32 * (1.0 - np.asarray(frozen, f32)))[None, :]
    return F.astype(f32)



# revision 9
# speedup vs baseline: 602.5194x; 602.5194x over previous
"""Trainium2 Bass kernel for all-pairs log-polar repulsion (gnn_message_passing).

Math: the reference's log-space distance chain collapses in linear space:
  exp(-ld) = 1/sqrt(dx^2+dy^2)  with x = r*(cos t + EPS*sign(cos t)), etc.
Row-sharded over 8 cores (512 query rows each). Each core streams 32 j-chunks
of 128 nodes; per chunk computes a [128j x 512i] force tile and reduces over j
with PE matmuls into PSUM:
  out0 = sum_j s_j*g_ij
  out1 = sum_j s_j*(ell_j - m)*g_ij       (m = mean ell, reduces cancellation)
  outq = sum_j s_j*g_ij*(t_ij - pi) - tau*sum_j s_j*g_ij*P + tau*sum_j s_j*g_ij*M
         with t_ij = th_j - th_i + pi, P = [t >= tau], M = [t < 0]
       = sum_j s_j*g_ij*wrap(th_j - th_i)   (exact jnp.mod wrap)
Host assembles F_ell = s_i*(out1 - (ell_i - m)*out0), F_th = s_i*outq.
The cutoff mask is applied on f = 1/sqrt(d2) directly ([f >= phi^-2] == [d2 <= phi^4]).
Diagonal self-pairs are zeroed with affine_select on the 4 local diag chunks
(per-core chunk permutation puts them at local chunks 0..3).
x/y/theta broadcast rows are expanded on-device by PE matmul against ones.
Engines: ACT sqx/sqy/f; Pool d2 (+diag mask); DVE g/gd/P/M; PE 4 matmuls.
"""

import sys

sys.path.insert(0, "/opt/trn_rl_repo")

from contextlib import ExitStack

import numpy as np

import concourse.bacc as bacc
import concourse.bass as bass
import concourse.mybir as mybir
import concourse.tile as tile

N = 4096
NCORES = 8
IPC = N // NCORES  # 512 rows per core
NJC = N // 128  # 32 j-chunks of 128
EPS = np.float32(1e-10)
PHI = (1.0 + np.sqrt(5.0)) / 2.0
TAU32 = float(np.float32(2.0 * np.pi))
PI32 = float(np.float32(np.pi))
CUT2 = float(np.float32(PHI**4))  # dist^2 cutoff = phi^4
INVF = float(np.float32(PHI**-2))  # f cutoff = 1/phi^2  ([f>=INVF] == [d2<=CUT2])

VARIANT = "pm"

NCOLS = 7 * NJC  # negx | negy | thj | scol | w2 (2 cols per chunk)
NROWS = 3 * IPC  # x | y | pi - theta rows

_cache = {}


def _build(variant=VARIANT):
    f32 = mybir.dt.float32
    bf16 = mybir.dt.bfloat16
    AF = mybir.ActivationFunctionType
    OP = mybir.AluOpType
    nc = bacc.Bacc()

    d_cols = nc.declare_dram_parameter("cols", [128, NCOLS], f32, isOutput=False)
    d_rows = nc.declare_dram_parameter("rows", [1, NROWS], f32, isOutput=False)
    d_out = nc.declare_dram_parameter("out", [3, IPC], f32, isOutput=True)

    with tile.TileContext(nc) as tc, ExitStack() as ctx:
        const = ctx.enter_context(tc.tile_pool(name="const", bufs=1))
        p_sqx = ctx.enter_context(tc.tile_pool(name="sqx", bufs=2))
        p_sqy = ctx.enter_context(tc.tile_pool(name="sqy", bufs=2))
        p_d2 = ctx.enter_context(tc.tile_pool(name="d2", bufs=2))
        p_f = ctx.enter_context(tc.tile_pool(name="f", bufs=2))
        p_g = ctx.enter_context(tc.tile_pool(name="g", bufs=2))
        p_gd = ctx.enter_context(tc.tile_pool(name="gd", bufs=2))
        p_P = ctx.enter_context(tc.tile_pool(name="P", bufs=2))
        p_M = ctx.enter_context(tc.tile_pool(name="M", bufs=2))
        psum = ctx.enter_context(tc.tile_pool(name="psum", bufs=1, space="PSUM"))

        t_cols = const.tile([128, NCOLS], f32)
        nc.gpsimd.dma_start(t_cols[:], d_cols[:])
        t_rows = const.tile([1, NROWS], f32)
        nc.gpsimd.dma_start(t_rows[:], d_rows[:])

        negx = t_cols[:, 0:NJC]
        negy = t_cols[:, NJC : 2 * NJC]
        thj = t_cols[:, 2 * NJC : 3 * NJC]
        scol = t_cols[:, 3 * NJC : 4 * NJC]
        w2 = t_cols[:, 4 * NJC : 6 * NJC]
        # spare region 6*NJC:7*NJC unused (alignment/padding)

        t_ones = const.tile([1, 128], f32)
        nc.gpsimd.memset(t_ones[:], 1.0)

        # broadcast x/y/(pi-theta) rows to 128 partitions via PE
        pb_x = psum.tile([128, IPC], f32)
        pb_y = psum.tile([128, IPC], f32)
        pb_t = psum.tile([128, IPC], f32)
        nc.tensor.matmul(pb_x[:], t_ones[:], t_rows[0:1, 0:IPC], start=True, stop=True)
        nc.tensor.matmul(
            pb_y[:], t_ones[:], t_rows[0:1, IPC : 2 * IPC], start=True, stop=True
        )
        nc.tensor.matmul(
            pb_t[:], t_ones[:], t_rows[0:1, 2 * IPC : 3 * IPC], start=True, stop=True
        )
        # DVE reads SBUF faster than PSUM; ACT reads PSUM fine.
        thrm2 = const.tile([128, IPC], f32)  # pi - theta_i rows, for DVE
        nc.vector.tensor_copy(thrm2[:], pb_t[:])

        # derived per-chunk scalars and bf16 weights (tiny one-time ops)
        thjmp = const.tile([128, NJC], f32)  # th_j - pi
        nc.vector.tensor_scalar(thjmp[:], thj, -PI32, None, op0=OP.add)
        tauthj = const.tile([128, NJC], f32)  # tau - th_j
        nc.vector.tensor_scalar(tauthj[:], thj, -1.0, TAU32, op0=OP.mult, op1=OP.add)
        negthj = const.tile([128, NJC], f32)  # -th_j
        nc.vector.tensor_scalar(negthj[:], thj, -1.0, None, op0=OP.mult)
        swq = const.tile([128, NJC], bf16)  # s_j
        nc.vector.tensor_copy(swq[:], scol)
        swm = const.tile([128, NJC], bf16)  # -tau*s_j
        nc.vector.tensor_scalar(swm[:], scol, -TAU32, None, op0=OP.mult)
        swp = const.tile([128, NJC], bf16)  # +tau*s_j
        nc.vector.tensor_scalar(swp[:], scol, TAU32, None, op0=OP.mult)

        psum2 = psum.tile([2, IPC], f32)
        psumq = psum.tile([1, IPC], f32)

        for c in range(NJC):
            first, last = c == 0, c == NJC - 1
            sqx = p_sqx.tile([128, IPC], f32)
            nc.scalar.activation(sqx[:], pb_x[:], AF.Square, bias=negx[:, c : c + 1])
            sqy = p_sqy.tile([128, IPC], f32)
            nc.scalar.activation(sqy[:], pb_y[:], AF.Square, bias=negy[:, c : c + 1])
            d2 = p_d2.tile([128, IPC], f32)
            nc.gpsimd.tensor_tensor(d2[:], sqx[:], sqy[:], op=OP.add)
            f = p_f.tile([128, IPC], f32)
            nc.scalar.activation(f[:], d2[:], AF.Abs_reciprocal_sqrt)
            g = p_g.tile([128, IPC], f32)
            nc.vector.scalar_tensor_tensor(
                g[:], f[:], INVF, f[:], op0=OP.is_ge, op1=OP.mult
            )
            if c < 4:  # local diag chunk: zero column i == 128*c + p
                g2 = p_g.tile([128, IPC], f32)
                nc.gpsimd.affine_select(
                    g2[:],
                    g[:],
                    pattern=[[1, IPC]],
                    compare_op=OP.not_equal,
                    fill=0.0,
                    base=-128 * c,
                    channel_multiplier=-1,
                )
                g = g2
            gd = p_gd.tile([128, IPC], bf16)
            nc.vector.scalar_tensor_tensor(
                gd[:], thrm2[:], thjmp[:, c : c + 1], g[:], op0=OP.add, op1=OP.mult
            )
            P = p_P.tile([128, IPC], bf16)
            nc.vector.scalar_tensor_tensor(
                P[:], thrm2[:], tauthj[:, c : c + 1], g[:], op0=OP.is_ge, op1=OP.mult
            )
            M = p_M.tile([128, IPC], bf16)
            nc.vector.scalar_tensor_tensor(
                M[:], thrm2[:], negthj[:, c : c + 1], g[:], op0=OP.is_lt, op1=OP.mult
            )
            nc.tensor.matmul(
                psum2[:], w2[:, 2 * c : 2 * c + 2], g[:], start=first, stop=last
            )
            nc.tensor.matmul(
                psumq[:], swq[:, c : c + 1], gd[:], start=first, stop=False
            )
            nc.tensor.matmul(
                psumq[:], swm[:, c : c + 1], P[:], start=False, stop=False
            )
            nc.tensor.matmul(
                psumq[:], swp[:, c : c + 1], M[:], start=False, stop=last
            )

        o2 = const.tile([2, IPC], f32)
        nc.vector.tensor_copy(o2[:], psum2[:])
        oq = const.tile([1, IPC], f32)
        nc.vector.tensor_copy(oq[:], psumq[:])
        nc.gpsimd.dma_start(d_out[0:2, :], o2[:])
        nc.gpsimd.dma_start(d_out[2:3, :], oq[:])

    if not nc.is_finalized():
        nc.finalize()
    return nc


def _host_prep(ell, theta, s, frozen, variant=VARIANT):
    f32 = np.float32
    ell = np.asarray(ell, f32)
    theta = np.asarray(theta, f32)
    s = np.asarray(s, f32)
    m = f32(ell.mean())
    c = np.cos(theta).astype(f32)
    sn = np.sin(theta).astype(f32)
    r = np.exp(ell).astype(f32)
    x = (r * (c + EPS * np.sign(c))).astype(f32)
    y = (r * (sn + EPS * np.sign(sn))).astype(f32)

    def cols(a):  # [N] -> [128, NJC], chunk c in column c
        return np.ascontiguousarray(a.reshape(NJC, 128).T)

    xc, yc, thc, sc, ec = cols(x), cols(y), cols(theta), cols(s), cols(ell)
    w2 = np.stack([sc, sc * (ec - m)], axis=2)  # [128, NJC, 2]
    pad = np.zeros((128, NJC), f32)

    cols_all = []
    rows_all = []
    for k in range(NCORES):
        perm = [(cc + 4 * k) % NJC for cc in range(NJC)]
        sl = slice(k * IPC, (k + 1) * IPC)
        payload = np.ascontiguousarray(
            np.concatenate(
                [
                    -xc[:, perm],
                    -yc[:, perm],
                    thc[:, perm],
                    sc[:, perm],
                    w2[:, perm, :].reshape(128, 2 * NJC),
                    pad,
                ],
                axis=1,
            ),
            dtype=f32,
        )
        assert payload.shape == (128, NCOLS)
        cols_all.append(payload)
        rows_all.append(
            np.concatenate([x[sl], y[sl], PI32 - theta[sl]]).astype(f32)[None, :]
        )
    return cols_all, rows_all, float(m)


def _assemble(ell, theta, s, frozen, outs, m):
    ell64 = np.asarray(ell, np.float64)
    s64 = np.asarray(s, np.float64)
    nf = 1.0 - np.asarray(frozen, np.float64)
    Fe = np.empty(N)
    Ft = np.empty(N)
    for k in range(NCORES):
        sl = slice(k * IPC, (k + 1) * IPC)
        o = np.asarray(outs[k], np.float64)
        Fe[sl] = o[1] - (ell64[sl] - m) * o[0]
        Ft[sl] = o[2]
    Fe *= s64 * nf
    Ft *= s64 * nf
    return np.stack([Fe, Ft]).astype(np.float32)


def _get_runner(variant=VARIANT):
    """Build nc once and return a cached jitted shard_map executor.

    Mirrors concourse.bass2jax.run_bass_via_pjrt but caches the jitted
    function so repeated kernel() calls don't retrace/recompile.
    """
    key = ("runner", variant)
    if key in _cache:
        return _cache[key]

    import jax
    from jax.sharding import Mesh, PartitionSpec
    from jax.experimental.shard_map import shard_map

    from concourse import bass2jax
    from concourse import mybir as _mybir

    bass2jax.install_neuronx_cc_hook()

    nc = _build(variant)

    in_names = []
    out_names = []
    out_avals = []
    zero_shapes = []
    assert nc.dbg_addr is None
    partition_name = nc.partition_id_tensor.name if nc.partition_id_tensor else None
    for alloc in nc.m.functions[0].allocations:
        if not isinstance(alloc, _mybir.MemoryLocationSet):
            continue
        name = alloc.memorylocations[0].name
        if alloc.kind == "ExternalInput":
            if name != partition_name:
                in_names.append(name)
        elif alloc.kind == "ExternalOutput":
            out_names.append(name)
            shape = tuple(alloc.tensor_shape)
            dtype = _mybir.dt.np(alloc.dtype)
            out_avals.append(jax.core.ShapedArray(shape, dtype))
            zero_shapes.append((shape, dtype))
    n_params = len(in_names)
    n_outs = len(out_avals)
    all_names = in_names + out_names
    if partition_name is not None:
        all_names = all_names + [partition_name]

    donate = tuple(range(n_params, n_params + n_outs))

    def _body(*args):
        operands = list(args)
        if partition_name is not None:
            operands.append(bass2jax.partition_id_tensor())
        outs = bass2jax._bass_exec_p.bind(
            *operands,
            out_avals=tuple(out_avals),
            in_names=tuple(all_names),
            out_names=tuple(out_names),
            lowering_input_output_aliases=(),
            sim_require_finite=True,
            sim_require_nnan=True,
            nc=nc,
        )
        return tuple(outs)

    devices = jax.devices()[:NCORES]
    mesh = Mesh(np.asarray(devices), ("core",))
    in_specs = (PartitionSpec("core"),) * (n_params + n_outs)
    out_specs = (PartitionSpec("core"),) * n_outs
    sharded = jax.jit(
        shard_map(
            _body, mesh=mesh, in_specs=in_specs, out_specs=out_specs, check_rep=False
        ),
        donate_argnums=donate,
        keep_unused=True,
    )

    runner = {
        "fn": sharded,
        "in_names": in_names,
        "out_names": out_names,
        "out_avals": out_avals,
        "zero_shapes": zero_shapes,
        "nc": nc,
    }
    _cache[key] = runner
    return runner


def run_device(ell, theta, s, frozen, variant=VARIANT):
    runner = _get_runner(variant)
    cols_all, rows_all, m = _host_prep(ell, theta, s, frozen, variant)
    in_map = {"cols": cols_all, "rows": rows_all}
    concat_in = [np.concatenate(in_map[name], axis=0) for name in runner["in_names"]]
    concat_zeros = [
        np.zeros((NCORES * shape[0],) + tuple(shape[1:]), dtype)
        for shape, dtype in runner["zero_shapes"]
    ]
    out_arrs = runner["fn"](*concat_in, *concat_zeros)
    oi = runner["out_names"].index("out")
    shape = runner["out_avals"][oi].shape
    outs = np.asarray(out_arrs[oi]).reshape(NCORES, *shape)
    return _assemble(ell, theta, s, frozen, outs, m)


def kernel(ell, theta, s, frozen):
    return run_device(ell, theta, s, frozen)
